# revision 4
# baseline (speedup 1.0000x reference)
"""Trainium2 Bass kernel for CausalTensionGraphLayer.

Math (reference factorization, with the wv/m2 merge folded on host):
  ac  = x @ [w1a | w1c] + [b1 | 0]      [grid, D]   (chunks 0-3 = a, 4-7 = c)
  u'  = 0.5 * (x @ P + qb)              [grid, D]   P = wv_w @ merge_w[D:],
                                                    qb = wv_b @ merge_w[D:]
  hid_w  = silu(a[t] + c[t-w-1])                    (c is 0 for t-w-1 < 0)
  th_w   = tanh((hid_w @ w2 + b2)/2)                (so tau = 0.5*(1+th))
  y[t]   = x[t] @ m1 + sum_w (1+th_w[t]) * u'[t-w-1] + merge_b
  out    = LayerNorm(y) * gamma + beta

Key identities: msg @ m2 = sum_w tau_w * (vzb @ m2) because tau_w[t] is a
per-token scalar (kills a full D x D matmul phase), and sigmoid(z) =
0.5*(1+tanh(z/2)) so the whole kernel fits the silu_and_others activation
table (Silu/Tanh/Copy/Square/Identity - no mid-kernel table switch).  The
0.5 folds into the u eviction, the +1 into the scalar_tensor_tensor gate
muls: zero extra instructions.

Gate weights w1 ship as fp8e3m4 scaled x64 (error feeds only the damped tau
path; validated 3.2e-3 end-to-end) which halves the startup-critical DMA.
The 1/64 descale folds into the ac eviction's activation scale.

Sharding: data-parallel over B*T = 8192 token rows, 1024 own tokens per core
plus a 4-row causal halo (zeros at batch boundaries).  No collectives.  All
inputs are pre-arranged on host into the exact SBUF tile layouts so every
DMA is a large contiguous-per-partition transfer.

Schedule: per token quarter, A (ac matmuls) -> gating front (hs adds + Silu,
overlapping U's matmuls) -> U -> tau matmuls + Tanh -> gate-sum gs (DVE
muls, GpSimd adds), with D(q-1) (merge + LayerNorm + store) interleaved one
quarter behind so the PE never idles and HAM stays at full clock.  gs
reaches the merge PSUM banks transposed via 128x128 identity matmuls that
accumulate on top of x@m1.  Eviction collects LN statistics via activation
accumulators (Copy for sum, in-place Square for sum-of-squares) over 2-bank
PSUM tiles; rstd via bit-trick + 2 Newton steps keeps everything in one
table set.  Warm-up matmuls at t=0 lift the PE clock gate during the DMA
fill.  Output is stored bf16 (within tolerance) to halve the drain.
"""

from contextlib import ExitStack

import numpy as np
import ml_dtypes

import concourse.bass as bass
import concourse.bacc as bacc
import concourse.tile as tile
from concourse import mybir
from concourse.bass_utils import run_bass_kernel_spmd

BF16 = ml_dtypes.bfloat16
FP8E3 = ml_dtypes.float8_e3m4

B, T, D = 2, 4096, 1024
H = D // 2
W = 4
EPS = 1e-5
NCORES = 8
NTOK = (B * T) // NCORES          # 1024 own tokens per core
HALO = W                          # 4
GRID = NTOK + HALO                # 1028
NQ = 4                            # token quarters per core
QT = NTOK // NQ                   # 256 own tokens per quarter
QG = QT + HALO                    # 260 grid cols per quarter
KD = D // 128                     # 8 K-chunks over D
MH = H // 128                     # 4 M-tiles over H
MD = D // 128                     # 8 M-tiles over D
NT = QT // 128                    # 2 token tiles per quarter
W1SCALE = 64.0                    # fp8e3m4 range scaling for w1

FP32 = mybir.dt.float32
I32 = mybir.dt.int32
BF = mybir.dt.bfloat16
F8 = mybir.dt.float8e3
AF = mybir.ActivationFunctionType
ALU = mybir.AluOpType


def build_nc(use_gamma_beta: bool, use_merge_b: bool):
    nc = bacc.Bacc(None, target_bir_lowering=False)

    xq = nc.dram_tensor("xq", [128, NQ, KD, QG], BF, kind="ExternalInput")
    wac = nc.dram_tensor("wac", [128, MD, KD, 128], F8, kind="ExternalInput")
    pw = nc.dram_tensor("pw", [128, MD, KD, 128], BF, kind="ExternalInput")
    m1 = nc.dram_tensor("m1", [128, 2, KD, 512], BF, kind="ExternalInput")
    w2rep = nc.dram_tensor("w2rep", [128, MH, 128], BF, kind="ExternalInput")
    iden = nc.dram_tensor("iden", [128, 128], BF, kind="ExternalInput")
    b1z = nc.dram_tensor("b1z", [128, MD], FP32, kind="ExternalInput")
    qbr = nc.dram_tensor("qbr", [128, MD], FP32, kind="ExternalInput")
    b2h = nc.dram_tensor("b2h", [128, 1], FP32, kind="ExternalInput")
    if use_gamma_beta:
        gam = nc.dram_tensor("gam", [1, D], FP32, kind="ExternalInput")
        bet = nc.dram_tensor("bet", [1, D], FP32, kind="ExternalInput")
    if use_merge_b:
        mbt = nc.dram_tensor("mbt", [1, D], FP32, kind="ExternalInput")
    y = nc.dram_tensor("y", [NTOK, D], BF, kind="ExternalOutput")

    with tile.TileContext(nc) as tc, ExitStack() as ctx:
        persist = ctx.enter_context(tc.tile_pool(name="persist", bufs=1))
        acpool = ctx.enter_context(tc.tile_pool(name="acpool", bufs=2))
        gspool = ctx.enter_context(tc.tile_pool(name="gspool", bufs=2))
        mpool = ctx.enter_context(tc.tile_pool(name="mpool", bufs=2))
        opool = ctx.enter_context(tc.tile_pool(name="opool", bufs=2))
        ps_acc = ctx.enter_context(tc.tile_pool(name="ps_acc", bufs=3, space="PSUM"))
        ps_log = ctx.enter_context(tc.tile_pool(name="ps_log", bufs=1, space="PSUM"))
        ps_y = ctx.enter_context(tc.tile_pool(name="ps_y", bufs=2, space="PSUM"))

        # ---- persistent tiles (SBUF layouts match DRAM exactly) ----------
        xq_sb = persist.tile([128, NQ, KD, QG], BF, tag="xq")
        wac_sb = persist.tile([128, MD, KD, 128], F8, tag="wac")
        pw_sb = persist.tile([128, MD, KD, 128], BF, tag="pw")
        m1_sb = persist.tile([128, 2, KD, 512], BF, tag="m1")
        w2rep_sb = persist.tile([128, MH, 128], BF, tag="w2rep")
        iden_sb = persist.tile([128, 128], BF, tag="iden")
        b1z_sb = persist.tile([128, MD], FP32, tag="b1z")
        qb_sb = persist.tile([128, MD], FP32, tag="qb")
        b2h_sb = persist.tile([128, 1], FP32, tag="b2h")

        # Input DMAs split across the two HWDGE rings (sync/scalar) in PE
        # consumption order; xq quarter 0 split so A(0) can start early.
        for mc in range(4):
            nc.sync.dma_start(
                out=wac_sb[:, 2 * mc:2 * mc + 2], in_=wac[:, 2 * mc:2 * mc + 2]
            )
        nc.scalar.dma_start(out=xq_sb[:, 0, 0:4], in_=xq[:, 0, 0:4])
        nc.scalar.dma_start(out=xq_sb[:, 0, 4:KD], in_=xq[:, 0, 4:KD])
        nc.scalar.dma_start(out=pw_sb[:, 0:2], in_=pw[:, 0:2])
        nc.scalar.dma_start(out=pw_sb[:, 2:4], in_=pw[:, 2:4])
        nc.scalar.dma_start(out=w2rep_sb, in_=w2rep[:, :])
        nc.scalar.dma_start(out=b1z_sb, in_=b1z[:, :])
        nc.scalar.dma_start(out=qb_sb, in_=qbr[:, :])
        nc.scalar.dma_start(out=b2h_sb, in_=b2h[:, :])
        nc.scalar.dma_start(out=iden_sb, in_=iden[:, :])
        if use_gamma_beta:
            gam_sb = persist.tile([128, D], FP32, tag="gam")
            nc.scalar.dma_start(out=gam_sb, in_=gam.partition_broadcast(128))
            bet_sb = persist.tile([128, D], FP32, tag="bet")
            nc.scalar.dma_start(out=bet_sb, in_=bet.partition_broadcast(128))
        if use_merge_b:
            mb_sb = persist.tile([128, D], FP32, tag="mb")
            nc.scalar.dma_start(out=mb_sb, in_=mbt.partition_broadcast(128))
        nc.sync.dma_start(out=xq_sb[:, 1], in_=xq[:, 1])
        nc.scalar.dma_start(out=pw_sb[:, 4:6], in_=pw[:, 4:6])
        nc.scalar.dma_start(out=pw_sb[:, 6:8], in_=pw[:, 6:8])
        nc.sync.dma_start(out=xq_sb[:, 2], in_=xq[:, 2])
        nc.scalar.dma_start(out=xq_sb[:, 3], in_=xq[:, 3])
        nc.sync.dma_start(out=m1_sb[:, 0], in_=m1[:, 0])
        nc.scalar.dma_start(out=m1_sb[:, 1], in_=m1[:, 1])

        magic_sb = persist.tile([128, 1], I32, tag="magic")
        nc.vector.memset(magic_sb, 0x5F3759DF)
        one_i = persist.tile([128, 1], I32, tag="onei")
        nc.vector.memset(one_i, 1)

        # ---- HAM warm-up: dummy matmuls while the first inputs stream ----
        warm_sb = persist.tile([128, 512], BF, tag="warm")
        nc.gpsimd.memset(warm_sb, 0)
        warm_ps = ps_log.tile([128, 512], FP32, tag="logit")
        NWARM = 7
        for i in range(NWARM):
            nc.tensor.matmul(
                warm_ps, warm_sb[:, 0:128], warm_sb,
                start=(i == 0), stop=(i == NWARM - 1),
            )

        # ---- main pipeline ----------------------------------------------
        def bcast(tauq, w):
            s = tauq[:, w, :]
            return bass.AP(
                tensor=s.tensor, offset=s.offset,
                ap=[s.ap[0], [0, MD], s.ap[1]],
            )

        gsqs = []

        def emit_D(q):
            g0 = q * QT
            gsq = gsqs[q]
            srow = mpool.tile([128, NT], FP32, tag="srow")
            sqs = mpool.tile([128, NT], FP32, tag="sqs")
            ysb = []
            for tt in range(NT):
                yps = ps_y.tile([128, 2, 512], FP32, tag="y")
                for half in range(2):
                    for k in range(KD):
                        nc.tensor.matmul(
                            yps[:, half, :],
                            xq_sb[:, q, k, HALO + 128 * tt:HALO + 128 * tt + 128],
                            m1_sb[:, half, k, :],
                            start=(k == 0),
                            stop=False,
                        )
                    # gs arrives transposed via identity matmuls, accumulated
                    # into the same banks (gated message + x@m1 in one go).
                    for mm in range(4):
                        m = half * 4 + mm
                        nc.tensor.matmul(
                            yps[:, half, mm * 128:(mm + 1) * 128],
                            gsq[:, m, 128 * tt:128 * tt + 128],
                            iden_sb,
                            start=False,
                            stop=(mm == 3),
                        )
                if use_merge_b:
                    nc.vector.tensor_add(
                        yps, yps, mb_sb.rearrange("p (a b) -> p a b", a=2)
                    )
                yt = opool.tile([128, 2, 512], FP32, tag="ysb")
                ysb.append(yt)
                nc.scalar.activation(
                    out=yt, in_=yps, func=AF.Copy,
                    accum_out=srow[:, tt:tt + 1],
                )
                nc.scalar.activation(   # in-place: yps is dead afterwards
                    out=yps, in_=yps, func=AF.Square,
                    accum_out=sqs[:, tt:tt + 1],
                )
            # LayerNorm finalize; rstd via bit-trick seed + 2 Newton steps.
            mean = mpool.tile([128, NT], FP32, tag="mean")
            nc.vector.tensor_scalar_mul(mean, srow, 1.0 / D)
            m2e = mpool.tile([128, NT], FP32, tag="m2e")
            nc.vector.scalar_tensor_tensor(   # mean^2 - eps
                out=m2e, in0=mean, scalar=1.0, in1=mean,
                op0=ALU.mult, op1=ALU.mult,
            )
            nc.vector.tensor_scalar_add(m2e, m2e, -EPS)
            veps = mpool.tile([128, NT], FP32, tag="veps")
            nc.vector.scalar_tensor_tensor(   # q/D - (mean^2 - eps)
                out=veps, in0=sqs, scalar=1.0 / D, in1=m2e,
                op0=ALU.mult, op1=ALU.subtract,
            )
            rbits = mpool.tile([128, NT], I32, tag="rbits")
            nc.vector.tensor_scalar(
                out=rbits, in0=veps.bitcast(I32), scalar1=one_i[:, 0:1],
                scalar2=None, op0=ALU.arith_shift_right,
            )
            nc.vector.tensor_tensor(
                out=rbits, in0=magic_sb.to_broadcast([128, NT]), in1=rbits,
                op=ALU.subtract,
            )
            rstd = rbits.bitcast(FP32)
            for _ in range(2):
                nt1 = mpool.tile([128, NT], FP32, tag="nt1")
                nc.vector.tensor_mul(nt1, rstd, rstd)
                nc.vector.tensor_mul(nt1, nt1, veps)
                nc.vector.tensor_scalar(
                    out=nt1, in0=nt1, scalar1=-0.5, scalar2=1.5,
                    op0=ALU.mult, op1=ALU.add,
                )
                nc.vector.tensor_mul(rstd, rstd, nt1)
            for tt in range(NT):
                tok0 = g0 + 128 * tt
                ybf = opool.tile([128, D], BF, tag="ybf")
                ytf = ysb[tt].rearrange("p a b -> p (a b)")
                if use_gamma_beta:
                    nc.vector.tensor_scalar(
                        out=ytf, in0=ytf, scalar1=mean[:, tt:tt + 1],
                        scalar2=rstd[:, tt:tt + 1],
                        op0=ALU.subtract, op1=ALU.mult,
                    )
                    nc.vector.tensor_mul(ytf, ytf, gam_sb)
                    nc.vector.tensor_add(ybf, ytf, bet_sb)
                else:
                    nc.vector.tensor_scalar(
                        out=ybf, in0=ytf, scalar1=mean[:, tt:tt + 1],
                        scalar2=rstd[:, tt:tt + 1],
                        op0=ALU.subtract, op1=ALU.mult,
                    )
                nc.sync.dma_start(out=y[tok0:tok0 + 128, :], in_=ybf)

        for q in range(NQ):
            # A(q): ac = (x @ [w1a|w1c]*64) / 64 + [b1|0] on the quarter grid
            acq = acpool.tile([128, MD, QG], BF, tag="acq")
            for m in range(MD):
                ps = ps_acc.tile([128, QG], FP32, tag="acc")
                for k in range(KD):
                    nc.tensor.matmul(
                        ps,
                        wac_sb[:, m, k, :],
                        xq_sb[:, q, k, :],
                        start=(k == 0),
                        stop=(k == KD - 1),
                    )
                nc.scalar.activation(
                    out=acq[:, m, :], in_=ps, func=AF.Identity,
                    bias=b1z_sb[:, m:m + 1], scale=1.0 / W1SCALE,
                )
            # gating front: hs = a + shift(c); Silu on ScalarE overlaps U
            hsss = []
            for p in range(W // 2):
                hs = mpool.tile([128, MH, 2, QT], BF, tag="hs")
                for wi in range(2):
                    w = 2 * p + wi
                    o = HALO - 1 - w
                    nc.vector.tensor_add(
                        hs[:, :, wi, :],
                        acq[:, 0:MH, HALO:HALO + QT],
                        acq[:, MH:MD, o:o + QT],
                    )
                hss = mpool.tile([128, MH, 2, QT], BF, tag="hss")
                nc.scalar.activation(out=hss, in_=hs, func=AF.Silu)
                hsss.append(hss)
            # U(q): u' = 0.5*(x @ P + qb) on the quarter grid
            uq = acpool.tile([128, MD, QG], BF, tag="uq")
            for m in range(MD):
                ps = ps_acc.tile([128, QG], FP32, tag="acc")
                for k in range(KD):
                    nc.tensor.matmul(
                        ps,
                        pw_sb[:, m, k, :],
                        xq_sb[:, q, k, :],
                        start=(k == 0),
                        stop=(k == KD - 1),
                    )
                nc.vector.tensor_scalar(
                    out=uq[:, m, :], in0=ps, scalar1=qb_sb[:, m:m + 1],
                    scalar2=0.5, op0=ALU.add, op1=ALU.mult,
                )
            # tau: th = tanh((hid @ w2 + b2)/2), pre-broadcast via w2rep
            tauq = mpool.tile([128, W, QT], BF, tag="tauq")
            for p in range(W // 2):
                pl = ps_log.tile([128, 2 * QT], FP32, tag="logit")
                for k in range(MH):
                    nc.tensor.matmul(
                        pl,
                        w2rep_sb[:, k, :],
                        hsss[p][:, k, :, :],
                        start=(k == 0),
                        stop=(k == MH - 1),
                    )
                nc.scalar.activation(
                    out=tauq[:, 2 * p:2 * p + 2, :],
                    in_=pl.rearrange("p (a b) -> p a b", a=2),
                    func=AF.Tanh,
                    bias=b2h_sb[:, 0:1], scale=0.5,
                )
            # gate-sum gs = sum_w (1+th_w) * u'_w: muls on DVE, adds GpSimd
            gsq = gspool.tile([128, MD, QT], BF, tag="gsq")
            gsqs.append(gsq)
            pt = []
            for w in range(W):
                o = HALO - 1 - w
                t = mpool.tile([128, MD, QT], BF, tag="pt", bufs=4)
                nc.vector.scalar_tensor_tensor(
                    out=t, in0=bcast(tauq, w), scalar=1.0,
                    in1=uq[:, :, o:o + QT], op0=ALU.add, op1=ALU.mult,
                )
                pt.append(t)
                if w == 1:
                    m01 = mpool.tile([128, MD, QT], BF, tag="pt", bufs=4)
                    nc.gpsimd.tensor_add(m01, pt[0], pt[1])
            nc.gpsimd.tensor_add(pt[3], pt[2], pt[3])
            nc.gpsimd.tensor_add(gsq, m01, pt[3])
            if q >= 1:
                emit_D(q - 1)
        emit_D(NQ - 1)
    nc.compile()
    return nc


_CACHE: dict = {}


def _get_nc(use_gamma_beta: bool, use_merge_b: bool):
    key = (use_gamma_beta, use_merge_b)
    if key not in _CACHE:
        _CACHE[key] = build_nc(use_gamma_beta, use_merge_b)
    return _CACHE[key]


def kernel(x, w1, b1, w2, b2, wv_w, wv_b, merge_w, merge_b, gamma, beta):
    x = np.asarray(x, dtype=np.float32)
    w1 = np.asarray(w1, dtype=np.float32)
    b1 = np.asarray(b1, dtype=np.float32)
    w2 = np.asarray(w2, dtype=np.float32)
    b2 = np.asarray(b2, dtype=np.float32)
    wv_w = np.asarray(wv_w, dtype=np.float32)
    wv_b = np.asarray(wv_b, dtype=np.float32)
    merge_w = np.asarray(merge_w, dtype=np.float32)
    merge_b = np.asarray(merge_b, dtype=np.float32)
    gamma = np.asarray(gamma, dtype=np.float32)
    beta = np.asarray(beta, dtype=np.float32)

    use_gamma_beta = not (np.all(gamma == 1.0) and np.all(beta == 0.0))
    use_merge_b = bool(np.any(merge_b != 0.0))
    nc = _get_nc(use_gamma_beta, use_merge_b)

    m1f = merge_w[:D]
    m2f = merge_w[D:]
    P = wv_w @ m2f                          # fold wv and merge projections
    qb = wv_b @ m2f

    wac_h = np.ascontiguousarray(
        (np.concatenate([w1[:D], w1[D:]], axis=1) * W1SCALE)
        .reshape(KD, 128, MD, 128).transpose(1, 2, 0, 3)
    ).astype(FP8E3)
    pw_h = np.ascontiguousarray(
        P.reshape(KD, 128, MD, 128).transpose(1, 2, 0, 3)
    ).astype(BF16)
    m1_h = np.ascontiguousarray(
        m1f.reshape(KD, 128, 2, 512).transpose(1, 2, 0, 3)
    ).astype(BF16)
    w2_h = np.ascontiguousarray(
        np.broadcast_to(w2.reshape(MH, 128, 1), (MH, 128, 128)).transpose(1, 0, 2)
    ).astype(BF16)
    b1zv = np.concatenate([b1, np.zeros(D - H, np.float32)])

    shared = {
        "wac": wac_h,
        "pw": pw_h,
        "m1": m1_h,
        "w2rep": w2_h,
        "iden": np.eye(128, dtype=BF16),
        "b1z": np.ascontiguousarray(b1zv.reshape(MD, 128).T.astype(np.float32)),
        "qbr": np.ascontiguousarray(qb.astype(np.float32).reshape(MD, 128).T),
        "b2h": np.full((128, 1), 0.5 * float(b2[0]), np.float32),
    }
    if use_gamma_beta:
        shared["gam"] = gamma.reshape(1, D)
        shared["bet"] = beta.reshape(1, D)
    if use_merge_b:
        shared["mbt"] = merge_b.reshape(1, D)

    x2T = np.ascontiguousarray(x.reshape(B * T, D).astype(BF16).T)  # [D, B*T]
    in_maps = []
    for c in range(NCORES):
        t0 = c * NTOK
        xsT = np.zeros((D, GRID), BF16)
        xsT[:, HALO:] = x2T[:, t0:t0 + NTOK]
        if t0 % T != 0:  # halo stays inside the same batch element
            xsT[:, :HALO] = x2T[:, t0 - HALO:t0]
        xk = xsT.reshape(KD, 128, GRID)
        xq_h = np.empty((128, NQ, KD, QG), BF16)
        for q in range(NQ):
            xq_h[:, q] = xk[:, :, q * QT:q * QT + QG].transpose(1, 0, 2)
        m = dict(shared)
        m["xq"] = xq_h
        in_maps.append(m)

    res = run_bass_kernel_spmd(nc, in_maps, core_ids=list(range(NCORES)))
    out = np.concatenate([r["y"] for r in res.results], axis=0)
    return out.reshape(B, T, D).astype(np.float32)


# revision 7
# speedup vs baseline: 1.3906x; 1.3906x over previous
"""Trainium2 Bass kernel for CausalTensionGraphLayer.

Math (reference factorization, with the wv/m2 merge folded on host):
  ac  = x @ [w1a | w1c] + [b1 | 0]      [grid, D]   (chunks 0-3 = a, 4-7 = c)
  u'  = 0.5 * (x @ P + qb)              [grid, D]   P = wv_w @ merge_w[D:],
                                                    qb = wv_b @ merge_w[D:]
  hid_w  = silu(a[t] + c[t-w-1])                    (c is 0 for t-w-1 < 0)
  th_w   = tanh((hid_w @ w2 + b2)/2)                (so tau = 0.5*(1+th))
  y[t]   = x[t] @ m1 + sum_w (1+th_w[t]) * u'[t-w-1] + merge_b
  out    = LayerNorm(y) * gamma + beta

Key identities: msg @ m2 = sum_w tau_w * (vzb @ m2) because tau_w[t] is a
per-token scalar (kills a full D x D matmul phase), and sigmoid(z) =
0.5*(1+tanh(z/2)) so the whole kernel fits the silu_and_others activation
table (Silu/Tanh/Copy/Square/Identity - no mid-kernel table switch).  The
0.5 folds into the u eviction, the +1 into the scalar_tensor_tensor gate
muls: zero extra instructions.

Gate weights w1 ship as fp8e3m4 scaled x64 (error feeds only the damped tau
path; validated 3.2e-3 end-to-end) which halves the startup-critical DMA.
The 1/64 descale folds into the ac eviction's activation scale.

Sharding: data-parallel over B*T = 8192 token rows, 1024 own tokens per core
plus a 4-row causal halo (zeros at batch boundaries).  No collectives.  All
inputs are pre-arranged on host into the exact SBUF tile layouts so every
DMA is a large contiguous-per-partition transfer.

Schedule: per token quarter, A (ac matmuls) -> gating front (hs adds + Silu,
overlapping U's matmuls) -> U -> tau matmuls + Tanh -> gate-sum gs (DVE
muls, GpSimd adds), with D(q-1) (merge + LayerNorm + store) interleaved one
quarter behind so the PE never idles and HAM stays at full clock.  gs
reaches the merge PSUM banks transposed via 128x128 identity matmuls that
accumulate on top of x@m1.  Eviction collects LN statistics via activation
accumulators (Copy for sum, in-place Square for sum-of-squares) over 2-bank
PSUM tiles; rstd via bit-trick + 2 Newton steps keeps everything in one
table set.  Warm-up matmuls at t=0 lift the PE clock gate during the DMA
fill.  Output is stored bf16 (within tolerance) to halve the drain.
"""

from contextlib import ExitStack

import numpy as np
import ml_dtypes

import concourse.bass as bass
import concourse.bacc as bacc
import concourse.tile as tile
from concourse import mybir
from concourse.bass_utils import run_bass_kernel_spmd

BF16 = ml_dtypes.bfloat16
FP8E3 = ml_dtypes.float8_e3m4

B, T, D = 2, 4096, 1024
H = D // 2
W = 4
EPS = 1e-5
NCORES = 8
NTOK = (B * T) // NCORES          # 1024 own tokens per core
HALO = W                          # 4
GRID = NTOK + HALO                # 1028
NQ = 4                            # token quarters per core
QT = NTOK // NQ                   # 256 own tokens per quarter
QG = QT + HALO                    # 260 grid cols per quarter
KD = D // 128                     # 8 K-chunks over D
MH = H // 128                     # 4 M-tiles over H
MD = D // 128                     # 8 M-tiles over D
NT = QT // 128                    # 2 token tiles per quarter
W1SCALE = 64.0                    # fp8e3m4 range scaling for w1

FP32 = mybir.dt.float32
I32 = mybir.dt.int32
BF = mybir.dt.bfloat16
F8 = mybir.dt.float8e3
AF = mybir.ActivationFunctionType
ALU = mybir.AluOpType


def build_nc(use_gamma_beta: bool, use_merge_b: bool):
    nc = bacc.Bacc(None, target_bir_lowering=False)

    xq = nc.dram_tensor("xq", [128, NQ, KD, QG], BF, kind="ExternalInput")
    wac = nc.dram_tensor("wac", [128, MD, KD, 128], F8, kind="ExternalInput")
    pw = nc.dram_tensor("pw", [128, MD, KD, 128], BF, kind="ExternalInput")
    m1 = nc.dram_tensor("m1", [128, 2, KD, 512], BF, kind="ExternalInput")
    w2rep = nc.dram_tensor("w2rep", [128, MH, 128], BF, kind="ExternalInput")
    iden = nc.dram_tensor("iden", [128, 128], BF, kind="ExternalInput")
    b1z = nc.dram_tensor("b1z", [128, MD], FP32, kind="ExternalInput")
    qbr = nc.dram_tensor("qbr", [128, MD], FP32, kind="ExternalInput")
    b2h = nc.dram_tensor("b2h", [128, 1], FP32, kind="ExternalInput")
    if use_gamma_beta:
        gam = nc.dram_tensor("gam", [1, D], FP32, kind="ExternalInput")
        bet = nc.dram_tensor("bet", [1, D], FP32, kind="ExternalInput")
    if use_merge_b:
        mbt = nc.dram_tensor("mbt", [1, D], FP32, kind="ExternalInput")
    y = nc.dram_tensor("y", [NTOK, D], BF, kind="ExternalOutput")

    with tile.TileContext(nc) as tc, ExitStack() as ctx:
        persist = ctx.enter_context(tc.tile_pool(name="persist", bufs=1))
        acpool = ctx.enter_context(tc.tile_pool(name="acpool", bufs=2))
        gspool = ctx.enter_context(tc.tile_pool(name="gspool", bufs=2))
        mpool = ctx.enter_context(tc.tile_pool(name="mpool", bufs=2))
        opool = ctx.enter_context(tc.tile_pool(name="opool", bufs=2))
        ps_acc = ctx.enter_context(tc.tile_pool(name="ps_acc", bufs=3, space="PSUM"))
        ps_log = ctx.enter_context(tc.tile_pool(name="ps_log", bufs=1, space="PSUM"))
        ps_y = ctx.enter_context(tc.tile_pool(name="ps_y", bufs=2, space="PSUM"))

        # ---- persistent tiles (SBUF layouts match DRAM exactly) ----------
        xq_sb = persist.tile([128, NQ, KD, QG], BF, tag="xq")
        wac_sb = persist.tile([128, MD, KD, 128], F8, tag="wac")
        pw_sb = persist.tile([128, MD, KD, 128], BF, tag="pw")
        m1_sb = persist.tile([128, 2, KD, 512], BF, tag="m1")
        w2rep_sb = persist.tile([128, MH, 128], BF, tag="w2rep")
        iden_sb = persist.tile([128, 128], BF, tag="iden")
        b1z_sb = persist.tile([128, MD], FP32, tag="b1z")
        qb_sb = persist.tile([128, MD], FP32, tag="qb")
        b2h_sb = persist.tile([128, 1], FP32, tag="b2h")

        # Input DMAs split across the two HWDGE rings (sync/scalar) in PE
        # consumption order; xq quarter 0 split so A(0) can start early.
        for mc in range(4):
            nc.sync.dma_start(
                out=wac_sb[:, 2 * mc:2 * mc + 2], in_=wac[:, 2 * mc:2 * mc + 2]
            )
        nc.scalar.dma_start(out=xq_sb[:, 0, 0:4], in_=xq[:, 0, 0:4])
        nc.scalar.dma_start(out=xq_sb[:, 0, 4:KD], in_=xq[:, 0, 4:KD])
        nc.scalar.dma_start(out=pw_sb[:, 0:2], in_=pw[:, 0:2])
        nc.scalar.dma_start(out=pw_sb[:, 2:4], in_=pw[:, 2:4])
        nc.scalar.dma_start(out=w2rep_sb, in_=w2rep[:, :])
        nc.scalar.dma_start(out=b1z_sb, in_=b1z[:, :])
        nc.scalar.dma_start(out=qb_sb, in_=qbr[:, :])
        nc.scalar.dma_start(out=b2h_sb, in_=b2h[:, :])
        nc.scalar.dma_start(out=iden_sb, in_=iden[:, :])
        if use_gamma_beta:
            gam_sb = persist.tile([128, D], FP32, tag="gam")
            nc.scalar.dma_start(out=gam_sb, in_=gam.partition_broadcast(128))
            bet_sb = persist.tile([128, D], FP32, tag="bet")
            nc.scalar.dma_start(out=bet_sb, in_=bet.partition_broadcast(128))
        if use_merge_b:
            mb_sb = persist.tile([128, D], FP32, tag="mb")
            nc.scalar.dma_start(out=mb_sb, in_=mbt.partition_broadcast(128))
        nc.sync.dma_start(out=xq_sb[:, 1], in_=xq[:, 1])
        nc.scalar.dma_start(out=pw_sb[:, 4:6], in_=pw[:, 4:6])
        nc.scalar.dma_start(out=pw_sb[:, 6:8], in_=pw[:, 6:8])
        nc.sync.dma_start(out=xq_sb[:, 2], in_=xq[:, 2])
        nc.scalar.dma_start(out=xq_sb[:, 3], in_=xq[:, 3])
        nc.sync.dma_start(out=m1_sb[:, 0], in_=m1[:, 0])
        nc.scalar.dma_start(out=m1_sb[:, 1], in_=m1[:, 1])

        magic_sb = persist.tile([128, 1], I32, tag="magic")
        nc.vector.memset(magic_sb, 0x5F3759DF)
        one_i = persist.tile([128, 1], I32, tag="onei")
        nc.vector.memset(one_i, 1)

        # ---- HAM warm-up: dummy matmuls while the first inputs stream ----
        warm_sb = persist.tile([128, 512], BF, tag="warm")
        nc.gpsimd.memset(warm_sb, 0)
        warm_ps = ps_log.tile([128, 512], FP32, tag="logit")
        NWARM = 7
        for i in range(NWARM):
            nc.tensor.matmul(
                warm_ps, warm_sb[:, 0:128], warm_sb,
                start=(i == 0), stop=(i == NWARM - 1),
            )

        # ---- main pipeline ----------------------------------------------
        def bcast(tauq, w):
            s = tauq[:, w, :]
            return bass.AP(
                tensor=s.tensor, offset=s.offset,
                ap=[s.ap[0], [0, MD], s.ap[1]],
            )

        gsqs = []

        def emit_D(q):
            g0 = q * QT
            gsq = gsqs[q]
            srow = mpool.tile([128, NT], FP32, tag="srow")
            sqs = mpool.tile([128, NT], FP32, tag="sqs")
            ysb = []
            for tt in range(NT):
                yps = ps_y.tile([128, 2, 512], FP32, tag="y")
                for half in range(2):
                    for k in range(KD):
                        nc.tensor.matmul(
                            yps[:, half, :],
                            xq_sb[:, q, k, HALO + 128 * tt:HALO + 128 * tt + 128],
                            m1_sb[:, half, k, :],
                            start=(k == 0),
                            stop=False,
                        )
                    # gs arrives transposed via identity matmuls, accumulated
                    # into the same banks (gated message + x@m1 in one go).
                    for mm in range(4):
                        m = half * 4 + mm
                        nc.tensor.matmul(
                            yps[:, half, mm * 128:(mm + 1) * 128],
                            gsq[:, m, 128 * tt:128 * tt + 128],
                            iden_sb,
                            start=False,
                            stop=(mm == 3),
                        )
                if use_merge_b:
                    nc.vector.tensor_add(
                        yps, yps, mb_sb.rearrange("p (a b) -> p a b", a=2)
                    )
                yt = opool.tile([128, 2, 512], FP32, tag="ysb")
                ysb.append(yt)
                nc.scalar.activation(
                    out=yt, in_=yps, func=AF.Copy,
                    accum_out=srow[:, tt:tt + 1],
                )
                nc.scalar.activation(   # in-place: yps is dead afterwards
                    out=yps, in_=yps, func=AF.Square,
                    accum_out=sqs[:, tt:tt + 1],
                )
            # LayerNorm finalize; rstd via bit-trick seed + 2 Newton steps.
            mean = mpool.tile([128, NT], FP32, tag="mean")
            nc.vector.tensor_scalar_mul(mean, srow, 1.0 / D)
            m2e = mpool.tile([128, NT], FP32, tag="m2e")
            nc.vector.scalar_tensor_tensor(   # mean^2 - eps
                out=m2e, in0=mean, scalar=1.0, in1=mean,
                op0=ALU.mult, op1=ALU.mult,
            )
            nc.vector.tensor_scalar_add(m2e, m2e, -EPS)
            veps = mpool.tile([128, NT], FP32, tag="veps")
            nc.vector.scalar_tensor_tensor(   # q/D - (mean^2 - eps)
                out=veps, in0=sqs, scalar=1.0 / D, in1=m2e,
                op0=ALU.mult, op1=ALU.subtract,
            )
            rbits = mpool.tile([128, NT], I32, tag="rbits")
            nc.vector.tensor_scalar(
                out=rbits, in0=veps.bitcast(I32), scalar1=one_i[:, 0:1],
                scalar2=None, op0=ALU.arith_shift_right,
            )
            nc.vector.tensor_tensor(
                out=rbits, in0=magic_sb.to_broadcast([128, NT]), in1=rbits,
                op=ALU.subtract,
            )
            rstd = rbits.bitcast(FP32)
            for _ in range(2):
                nt1 = mpool.tile([128, NT], FP32, tag="nt1")
                nc.vector.tensor_mul(nt1, rstd, rstd)
                nc.vector.tensor_mul(nt1, nt1, veps)
                nc.vector.tensor_scalar(
                    out=nt1, in0=nt1, scalar1=-0.5, scalar2=1.5,
                    op0=ALU.mult, op1=ALU.add,
                )
                nc.vector.tensor_mul(rstd, rstd, nt1)
            for tt in range(NT):
                tok0 = g0 + 128 * tt
                ybf = opool.tile([128, D], BF, tag="ybf")
                ytf = ysb[tt].rearrange("p a b -> p (a b)")
                if use_gamma_beta:
                    nc.vector.tensor_scalar(
                        out=ytf, in0=ytf, scalar1=mean[:, tt:tt + 1],
                        scalar2=rstd[:, tt:tt + 1],
                        op0=ALU.subtract, op1=ALU.mult,
                    )
                    nc.vector.tensor_mul(ytf, ytf, gam_sb)
                    nc.vector.tensor_add(ybf, ytf, bet_sb)
                else:
                    nc.vector.tensor_scalar(
                        out=ybf, in0=ytf, scalar1=mean[:, tt:tt + 1],
                        scalar2=rstd[:, tt:tt + 1],
                        op0=ALU.subtract, op1=ALU.mult,
                    )
                nc.sync.dma_start(out=y[tok0:tok0 + 128, :], in_=ybf)

        for q in range(NQ):
            # A(q): ac = (x @ [w1a|w1c]*64) / 64 + [b1|0] on the quarter grid
            acq = acpool.tile([128, MD, QG], BF, tag="acq")
            for m in range(MD):
                ps = ps_acc.tile([128, QG], FP32, tag="acc")
                for k in range(KD):
                    nc.tensor.matmul(
                        ps,
                        wac_sb[:, m, k, :],
                        xq_sb[:, q, k, :],
                        start=(k == 0),
                        stop=(k == KD - 1),
                    )
                nc.scalar.activation(
                    out=acq[:, m, :], in_=ps, func=AF.Identity,
                    bias=b1z_sb[:, m:m + 1], scale=1.0 / W1SCALE,
                )
            # gating front: hs = a + shift(c); Silu on ScalarE overlaps U
            hsss = []
            for p in range(W // 2):
                hs = mpool.tile([128, MH, 2, QT], BF, tag="hs")
                for wi in range(2):
                    w = 2 * p + wi
                    o = HALO - 1 - w
                    nc.vector.tensor_add(
                        hs[:, :, wi, :],
                        acq[:, 0:MH, HALO:HALO + QT],
                        acq[:, MH:MD, o:o + QT],
                    )
                hss = mpool.tile([128, MH, 2, QT], BF, tag="hss")
                nc.scalar.activation(out=hss, in_=hs, func=AF.Silu)
                hsss.append(hss)
            # U(q): u' = 0.5*(x @ P + qb) on the quarter grid
            uq = acpool.tile([128, MD, QG], BF, tag="uq")
            for m in range(MD):
                ps = ps_acc.tile([128, QG], FP32, tag="acc")
                for k in range(KD):
                    nc.tensor.matmul(
                        ps,
                        pw_sb[:, m, k, :],
                        xq_sb[:, q, k, :],
                        start=(k == 0),
                        stop=(k == KD - 1),
                    )
                if m < 4:   # split evictions across ScalarE and DVE
                    nc.scalar.activation(
                        out=uq[:, m, :], in_=ps, func=AF.Identity,
                        bias=qb_sb[:, m:m + 1], scale=0.5,
                    )
                else:
                    nc.vector.tensor_scalar(
                        out=uq[:, m, :], in0=ps, scalar1=0.5,
                        scalar2=qb_sb[:, m:m + 1], op0=ALU.mult, op1=ALU.add,
                    )
            # tau: th = tanh((hid @ w2 + b2)/2), pre-broadcast via w2rep
            tauq = mpool.tile([128, W, QT], BF, tag="tauq")
            for p in range(W // 2):
                pl = ps_log.tile([128, 2 * QT], FP32, tag="logit")
                for k in range(MH):
                    nc.tensor.matmul(
                        pl,
                        w2rep_sb[:, k, :],
                        hsss[p][:, k, :, :],
                        start=(k == 0),
                        stop=(k == MH - 1),
                    )
                nc.scalar.activation(
                    out=tauq[:, 2 * p:2 * p + 2, :],
                    in_=pl.rearrange("p (a b) -> p a b", a=2),
                    func=AF.Tanh,
                    bias=b2h_sb[:, 0:1], scale=0.5,
                )
            # gate-sum gs = sum_w (1+th_w) * u'_w on DVE
            nc.vector.tensor_scalar_add(tauq, tauq, 1.0)
            gsq = gspool.tile([128, MD, QT], BF, tag="gsq")
            gsqs.append(gsq)
            pt = []
            for w in range(W):
                o = HALO - 1 - w
                t = mpool.tile([128, MD, QT], BF, tag="pt", bufs=4)
                nc.vector.tensor_mul(t, bcast(tauq, w), uq[:, :, o:o + QT])
                pt.append(t)
                if w == 1:
                    m01 = mpool.tile([128, MD, QT], BF, tag="pt", bufs=4)
                    nc.vector.tensor_add(m01, pt[0], pt[1])
            nc.vector.tensor_add(pt[3], pt[2], pt[3])
            nc.vector.tensor_add(gsq, m01, pt[3])
            if q >= 1:
                emit_D(q - 1)
        emit_D(NQ - 1)
    nc.compile()
    return nc


_CACHE: dict = {}


def _get_nc(use_gamma_beta: bool, use_merge_b: bool):
    key = (use_gamma_beta, use_merge_b)
    if key not in _CACHE:
        _CACHE[key] = build_nc(use_gamma_beta, use_merge_b)
    return _CACHE[key]


def kernel(x, w1, b1, w2, b2, wv_w, wv_b, merge_w, merge_b, gamma, beta):
    x = np.asarray(x, dtype=np.float32)
    w1 = np.asarray(w1, dtype=np.float32)
    b1 = np.asarray(b1, dtype=np.float32)
    w2 = np.asarray(w2, dtype=np.float32)
    b2 = np.asarray(b2, dtype=np.float32)
    wv_w = np.asarray(wv_w, dtype=np.float32)
    wv_b = np.asarray(wv_b, dtype=np.float32)
    merge_w = np.asarray(merge_w, dtype=np.float32)
    merge_b = np.asarray(merge_b, dtype=np.float32)
    gamma = np.asarray(gamma, dtype=np.float32)
    beta = np.asarray(beta, dtype=np.float32)

    use_gamma_beta = not (np.all(gamma == 1.0) and np.all(beta == 0.0))
    use_merge_b = bool(np.any(merge_b != 0.0))
    nc = _get_nc(use_gamma_beta, use_merge_b)

    m1f = merge_w[:D]
    m2f = merge_w[D:]
    P = wv_w @ m2f                          # fold wv and merge projections
    qb = wv_b @ m2f

    wac_h = np.ascontiguousarray(
        (np.concatenate([w1[:D], w1[D:]], axis=1) * W1SCALE)
        .reshape(KD, 128, MD, 128).transpose(1, 2, 0, 3)
    ).astype(FP8E3)
    pw_h = np.ascontiguousarray(
        P.reshape(KD, 128, MD, 128).transpose(1, 2, 0, 3)
    ).astype(BF16)
    m1_h = np.ascontiguousarray(
        m1f.reshape(KD, 128, 2, 512).transpose(1, 2, 0, 3)
    ).astype(BF16)
    w2_h = np.ascontiguousarray(
        np.broadcast_to(w2.reshape(MH, 128, 1), (MH, 128, 128)).transpose(1, 0, 2)
    ).astype(BF16)
    b1zv = np.concatenate([b1, np.zeros(D - H, np.float32)])

    shared = {
        "wac": wac_h,
        "pw": pw_h,
        "m1": m1_h,
        "w2rep": w2_h,
        "iden": np.eye(128, dtype=BF16),
        "b1z": np.ascontiguousarray(b1zv.reshape(MD, 128).T.astype(np.float32)),
        "qbr": np.ascontiguousarray(
            (0.5 * qb).astype(np.float32).reshape(MD, 128).T
        ),
        "b2h": np.full((128, 1), 0.5 * float(b2[0]), np.float32),
    }
    if use_gamma_beta:
        shared["gam"] = gamma.reshape(1, D)
        shared["bet"] = beta.reshape(1, D)
    if use_merge_b:
        shared["mbt"] = merge_b.reshape(1, D)

    x2T = np.ascontiguousarray(x.reshape(B * T, D).astype(BF16).T)  # [D, B*T]
    in_maps = []
    for c in range(NCORES):
        t0 = c * NTOK
        xsT = np.zeros((D, GRID), BF16)
        xsT[:, HALO:] = x2T[:, t0:t0 + NTOK]
        if t0 % T != 0:  # halo stays inside the same batch element
            xsT[:, :HALO] = x2T[:, t0 - HALO:t0]
        xk = xsT.reshape(KD, 128, GRID)
        xq_h = np.empty((128, NQ, KD, QG), BF16)
        for q in range(NQ):
            xq_h[:, q] = xk[:, :, q * QT:q * QT + QG].transpose(1, 0, 2)
        m = dict(shared)
        m["xq"] = xq_h
        in_maps.append(m)

    res = run_bass_kernel_spmd(nc, in_maps, core_ids=list(range(NCORES)))
    out = np.concatenate([r["y"] for r in res.results], axis=0)
    return out.reshape(B, T, D).astype(np.float32)


# revision 15
# speedup vs baseline: 1.4058x; 1.0110x over previous
"""Trainium2 Bass kernel for CausalTensionGraphLayer.

Math (reference factorization, with the wv/m2 merge folded on host):
  ac  = x @ [w1a | w1c] + [b1 | 0]      [grid, D]   (chunks 0-3 = a, 4-7 = c)
  u'  = 0.5 * (x @ P + qb)              [grid, D]   P = wv_w @ merge_w[D:],
                                                    qb = wv_b @ merge_w[D:]
  hid_w  = silu(a[t] + c[t-w-1])                    (c is 0 for t-w-1 < 0)
  th_w   = tanh((hid_w @ w2 + b2)/2)                (so tau = 0.5*(1+th))
  y[t]   = x[t] @ m1 + sum_w (1+th_w[t]) * u'[t-w-1] + merge_b
  out    = LayerNorm(y) * gamma + beta

Key identities: msg @ m2 = sum_w tau_w * (vzb @ m2) because tau_w[t] is a
per-token scalar (kills a full D x D matmul phase), and sigmoid(z) =
0.5*(1+tanh(z/2)) so the whole kernel fits the silu_and_others activation
table (Silu/Tanh/Copy/Square/Identity - no mid-kernel table switch).  The
0.5 folds into the u eviction, the +1 into the scalar_tensor_tensor gate
muls: zero extra instructions.

Gate weights w1 ship as fp8e3m4 scaled x64 (error feeds only the damped tau
path; validated 3.2e-3 end-to-end) which halves the startup-critical DMA.
The 1/64 descale folds into the ac eviction's activation scale.

Sharding: data-parallel over B*T = 8192 token rows, 1024 own tokens per core
plus a 4-row causal halo (zeros at batch boundaries).  No collectives.  All
inputs are pre-arranged on host into the exact SBUF tile layouts so every
DMA is a large contiguous-per-partition transfer.

Schedule: per token quarter, A (ac matmuls) -> gating front (hs adds + Silu,
overlapping U's matmuls) -> U -> tau matmuls + Tanh -> gate-sum gs (DVE
muls, GpSimd adds), with D(q-1) (merge + LayerNorm + store) interleaved one
quarter behind so the PE never idles and HAM stays at full clock.  gs
reaches the merge PSUM banks transposed via 128x128 identity matmuls that
accumulate on top of x@m1.  Eviction collects LN statistics via activation
accumulators (Copy for sum, in-place Square for sum-of-squares) over 2-bank
PSUM tiles; rstd via bit-trick + 2 Newton steps keeps everything in one
table set.  Warm-up matmuls at t=0 lift the PE clock gate during the DMA
fill.  Output is stored bf16 (within tolerance) to halve the drain.
"""

from contextlib import ExitStack

import numpy as np
import ml_dtypes

import concourse.bass as bass
import concourse.bacc as bacc
import concourse.tile as tile
from concourse import mybir
from concourse.bass_utils import run_bass_kernel_spmd

BF16 = ml_dtypes.bfloat16
FP8E3 = ml_dtypes.float8_e3m4

B, T, D = 2, 4096, 1024
H = D // 2
W = 4
EPS = 1e-5
NCORES = 8
NTOK = (B * T) // NCORES          # 1024 own tokens per core
HALO = W                          # 4
GRID = NTOK + HALO                # 1028
NQ = 4                            # token quarters per core
QT = NTOK // NQ                   # 256 own tokens per quarter
QG = QT + HALO                    # 260 grid cols per quarter
KD = D // 128                     # 8 K-chunks over D
MH = H // 128                     # 4 M-tiles over H
MD = D // 128                     # 8 M-tiles over D
NT = QT // 128                    # 2 token tiles per quarter
W1SCALE = 64.0                    # fp8e3m4 range scaling for w1

FP32 = mybir.dt.float32
I32 = mybir.dt.int32
BF = mybir.dt.bfloat16
F8 = mybir.dt.float8e3
AF = mybir.ActivationFunctionType
ALU = mybir.AluOpType
AX = mybir.AxisListType


def build_nc(use_gamma_beta: bool, use_merge_b: bool):
    nc = bacc.Bacc(None, target_bir_lowering=False)

    xq = nc.dram_tensor("xq", [128, NQ, KD, QG], BF, kind="ExternalInput")
    wac = nc.dram_tensor("wac", [128, MD, KD, 128], F8, kind="ExternalInput")
    pw = nc.dram_tensor("pw", [128, MD, KD, 128], BF, kind="ExternalInput")
    m1 = nc.dram_tensor("m1", [128, 2, KD, 512], BF, kind="ExternalInput")
    w2rep = nc.dram_tensor("w2rep", [128, MH, 128], BF, kind="ExternalInput")
    iden = nc.dram_tensor("iden", [128, 128], BF, kind="ExternalInput")
    # biases packed into one tensor: cols 0-7 = [b1|0], 8-15 = qb/2, 16 = b2/2
    bias = nc.dram_tensor("bias", [128, 2 * MD + 1], FP32, kind="ExternalInput")
    if use_gamma_beta:
        gam = nc.dram_tensor("gam", [1, D], FP32, kind="ExternalInput")
        bet = nc.dram_tensor("bet", [1, D], FP32, kind="ExternalInput")
    if use_merge_b:
        mbt = nc.dram_tensor("mbt", [1, D], FP32, kind="ExternalInput")
    y = nc.dram_tensor("y", [NTOK, D], BF, kind="ExternalOutput")

    with tile.TileContext(nc) as tc, ExitStack() as ctx:
        persist = ctx.enter_context(tc.tile_pool(name="persist", bufs=1))
        acpool = ctx.enter_context(tc.tile_pool(name="acpool", bufs=2))
        gspool = ctx.enter_context(tc.tile_pool(name="gspool", bufs=2))
        mpool = ctx.enter_context(tc.tile_pool(name="mpool", bufs=2))
        opool = ctx.enter_context(tc.tile_pool(name="opool", bufs=2))
        ps_acc = ctx.enter_context(tc.tile_pool(name="ps_acc", bufs=3, space="PSUM"))
        ps_log = ctx.enter_context(tc.tile_pool(name="ps_log", bufs=1, space="PSUM"))
        ps_y = ctx.enter_context(tc.tile_pool(name="ps_y", bufs=2, space="PSUM"))

        # ---- persistent tiles (SBUF layouts match DRAM exactly) ----------
        xq_sb = persist.tile([128, NQ, KD, QG], BF, tag="xq")
        wac_sb = persist.tile([128, MD, KD, 128], F8, tag="wac")
        pw_sb = persist.tile([128, MD, KD, 128], BF, tag="pw")
        m1_sb = persist.tile([128, 2, KD, 512], BF, tag="m1")
        w2rep_sb = persist.tile([128, MH, 128], BF, tag="w2rep")
        iden_sb = persist.tile([128, 128], BF, tag="iden")
        bias_sb = persist.tile([128, 2 * MD + 1], FP32, tag="bias")
        b1z_sb = bias_sb[:, 0:MD]
        qb_sb = bias_sb[:, MD:2 * MD]
        b2h_sb = bias_sb[:, 2 * MD:2 * MD + 1]

        # Input DMAs split across the two HWDGE rings (sync/scalar) in PE
        # consumption order; xq quarter 0 split so A(0) can start early.
        # Most issues sit on the sync engine, which has no compute to run.
        for mc in range(4):
            nc.sync.dma_start(
                out=wac_sb[:, 2 * mc:2 * mc + 2], in_=wac[:, 2 * mc:2 * mc + 2]
            )
        nc.scalar.dma_start(out=bias_sb, in_=bias[:, :])
        nc.scalar.dma_start(out=xq_sb[:, 0, 0:4], in_=xq[:, 0, 0:4])
        nc.scalar.dma_start(out=xq_sb[:, 0, 4:KD], in_=xq[:, 0, 4:KD])
        nc.scalar.dma_start(out=pw_sb[:, 0:2], in_=pw[:, 0:2])
        nc.sync.dma_start(out=pw_sb[:, 2:4], in_=pw[:, 2:4])
        nc.sync.dma_start(out=pw_sb[:, 4:6], in_=pw[:, 4:6])
        nc.scalar.dma_start(out=pw_sb[:, 6:8], in_=pw[:, 6:8])
        nc.sync.dma_start(out=xq_sb[:, 1], in_=xq[:, 1])
        nc.sync.dma_start(out=w2rep_sb, in_=w2rep[:, :])
        nc.sync.dma_start(out=iden_sb, in_=iden[:, :])
        if use_gamma_beta:
            gam_sb = persist.tile([128, D], FP32, tag="gam")
            nc.sync.dma_start(out=gam_sb, in_=gam.partition_broadcast(128))
            bet_sb = persist.tile([128, D], FP32, tag="bet")
            nc.sync.dma_start(out=bet_sb, in_=bet.partition_broadcast(128))
        if use_merge_b:
            mb_sb = persist.tile([128, D], FP32, tag="mb")
            nc.sync.dma_start(out=mb_sb, in_=mbt.partition_broadcast(128))
        nc.sync.dma_start(out=xq_sb[:, 2], in_=xq[:, 2])
        nc.sync.dma_start(out=xq_sb[:, 3], in_=xq[:, 3])
        nc.sync.dma_start(out=m1_sb[:, 0], in_=m1[:, 0])
        nc.scalar.dma_start(out=m1_sb[:, 1], in_=m1[:, 1])

        magic_sb = persist.tile([128, 1], I32, tag="magic")
        nc.vector.memset(magic_sb, 0x5F3759DF)
        one_i = persist.tile([128, 1], I32, tag="onei")
        nc.vector.memset(one_i, 1)

        # ---- HAM warm-up: dummy matmuls while the first inputs stream ----
        warm_sb = persist.tile([128, 512], BF, tag="warm")
        nc.gpsimd.memset(warm_sb, 0)
        warm_ps = ps_log.tile([128, 512], FP32, tag="logit")
        NWARM = 9
        for i in range(NWARM):
            nc.tensor.matmul(
                warm_ps, warm_sb[:, 0:128], warm_sb,
                start=(i == 0), stop=(i == NWARM - 1),
            )

        # ---- main pipeline ----------------------------------------------
        def bcast(tauq, w):
            s = tauq[:, w, :]
            return bass.AP(
                tensor=s.tensor, offset=s.offset,
                ap=[s.ap[0], [0, MD], s.ap[1]],
            )

        gsqs = []

        def emit_D(q):
            g0 = q * QT
            gsq = gsqs[q]
            srow = mpool.tile([128, NT, 2], FP32, tag="srow")
            sqs = mpool.tile([128, NT, 2], FP32, tag="sqs")
            ysb = []
            for tt in range(NT):
                yps = ps_y.tile([128, 2, 512], FP32, tag="y")
                yt = opool.tile([128, 2, 512], FP32, tag="ysb")
                ysb.append(yt)
                for half in range(2):
                    for k in range(KD):
                        nc.tensor.matmul(
                            yps[:, half, :],
                            xq_sb[:, q, k, HALO + 128 * tt:HALO + 128 * tt + 128],
                            m1_sb[:, half, k, :],
                            start=(k == 0),
                            stop=False,
                        )
                    # gs arrives transposed via identity matmuls, accumulated
                    # into the same banks (gated message + x@m1 in one go).
                    for mm in range(4):
                        m = half * 4 + mm
                        nc.tensor.matmul(
                            yps[:, half, mm * 128:(mm + 1) * 128],
                            gsq[:, m, 128 * tt:128 * tt + 128],
                            iden_sb,
                            start=False,
                            stop=(mm == 3),
                        )
                    if use_merge_b:
                        nc.vector.tensor_add(
                            yps[:, half, :], yps[:, half, :],
                            mb_sb[:, 512 * half:512 * half + 512],
                        )
                    # per-half eviction pipelines with the PE at bank level
                    nc.scalar.activation(
                        out=yt[:, half, :], in_=yps[:, half, :], func=AF.Copy,
                        accum_out=srow[:, tt, half:half + 1],
                    )
                    nc.scalar.activation(   # in-place: this bank dies here
                        out=yps[:, half, :], in_=yps[:, half, :], func=AF.Square,
                        accum_out=sqs[:, tt, half:half + 1],
                    )
            # LayerNorm finalize; rstd via bit-trick seed + 1 Newton step.
            ssum = mpool.tile([128, NT], FP32, tag="ssum")
            nc.vector.reduce_sum(out=ssum, in_=srow, axis=AX.X)
            qsum = mpool.tile([128, NT], FP32, tag="qsum")
            nc.vector.reduce_sum(out=qsum, in_=sqs, axis=AX.X)
            mean = mpool.tile([128, NT], FP32, tag="mean")
            nc.vector.tensor_scalar_mul(mean, ssum, 1.0 / D)
            m2e = mpool.tile([128, NT], FP32, tag="m2e")
            nc.vector.scalar_tensor_tensor(   # mean^2 - eps
                out=m2e, in0=mean, scalar=1.0, in1=mean,
                op0=ALU.mult, op1=ALU.mult,
            )
            nc.vector.tensor_scalar_add(m2e, m2e, -EPS)
            veps = mpool.tile([128, NT], FP32, tag="veps")
            nc.vector.scalar_tensor_tensor(   # q/D - (mean^2 - eps)
                out=veps, in0=qsum, scalar=1.0 / D, in1=m2e,
                op0=ALU.mult, op1=ALU.subtract,
            )
            rbits = mpool.tile([128, NT], I32, tag="rbits")
            nc.vector.tensor_scalar(
                out=rbits, in0=veps.bitcast(I32), scalar1=one_i[:, 0:1],
                scalar2=None, op0=ALU.arith_shift_right,
            )
            nc.vector.tensor_tensor(
                out=rbits, in0=magic_sb.to_broadcast([128, NT]), in1=rbits,
                op=ALU.subtract,
            )
            rstd = rbits.bitcast(FP32)
            for _ in range(1):
                nt1 = mpool.tile([128, NT], FP32, tag="nt1")
                nc.vector.tensor_mul(nt1, rstd, rstd)
                nc.vector.tensor_mul(nt1, nt1, veps)
                nc.vector.tensor_scalar(
                    out=nt1, in0=nt1, scalar1=-0.5, scalar2=1.5,
                    op0=ALU.mult, op1=ALU.add,
                )
                nc.vector.tensor_mul(rstd, rstd, nt1)
            for tt in range(NT):
                tok0 = g0 + 128 * tt
                ybf = opool.tile([128, D], BF, tag="ybf")
                ytf = ysb[tt].rearrange("p a b -> p (a b)")
                if use_gamma_beta:
                    nc.vector.tensor_scalar(
                        out=ytf, in0=ytf, scalar1=mean[:, tt:tt + 1],
                        scalar2=rstd[:, tt:tt + 1],
                        op0=ALU.subtract, op1=ALU.mult,
                    )
                    nc.vector.tensor_mul(ytf, ytf, gam_sb)
                    nc.vector.tensor_add(ybf, ytf, bet_sb)
                else:
                    nc.vector.tensor_scalar(
                        out=ybf, in0=ytf, scalar1=mean[:, tt:tt + 1],
                        scalar2=rstd[:, tt:tt + 1],
                        op0=ALU.subtract, op1=ALU.mult,
                    )
                nc.sync.dma_start(out=y[tok0:tok0 + 128, :], in_=ybf)

        for q in range(NQ):
            # A(q): ac = (x @ [w1a|w1c]*64) / 64 + [b1|0] on the quarter grid
            acq = acpool.tile([128, MD, QG], BF, tag="acq")
            for m in range(MD):
                ps = ps_acc.tile([128, QG], FP32, tag="acc")
                for k in range(KD):
                    nc.tensor.matmul(
                        ps,
                        wac_sb[:, m, k, :],
                        xq_sb[:, q, k, :],
                        start=(k == 0),
                        stop=(k == KD - 1),
                    )
                nc.scalar.activation(
                    out=acq[:, m, :], in_=ps, func=AF.Identity,
                    bias=b1z_sb[:, m:m + 1], scale=1.0 / W1SCALE,
                )
            # gating front: hs = a + shift(c); Silu on ScalarE overlaps U
            hsss = []
            for p in range(W // 2):
                hs = mpool.tile([128, MH, 2, QT], BF, tag="hs")
                for wi in range(2):
                    w = 2 * p + wi
                    o = HALO - 1 - w
                    nc.vector.tensor_add(
                        hs[:, :, wi, :],
                        acq[:, 0:MH, HALO:HALO + QT],
                        acq[:, MH:MD, o:o + QT],
                    )
                hss = mpool.tile([128, MH, 2, QT], BF, tag="hss")
                nc.scalar.activation(out=hss, in_=hs, func=AF.Silu)
                hsss.append(hss)
            # U(q): u' = 0.5*(x @ P + qb) on the quarter grid
            uq = acpool.tile([128, MD, QG], BF, tag="uq")
            for m in range(MD):
                ps = ps_acc.tile([128, QG], FP32, tag="acc")
                for k in range(KD):
                    nc.tensor.matmul(
                        ps,
                        pw_sb[:, m, k, :],
                        xq_sb[:, q, k, :],
                        start=(k == 0),
                        stop=(k == KD - 1),
                    )
                if m < 4:   # split evictions across ScalarE and DVE
                    nc.scalar.activation(
                        out=uq[:, m, :], in_=ps, func=AF.Identity,
                        bias=qb_sb[:, m:m + 1], scale=0.5,
                    )
                else:
                    nc.vector.tensor_scalar(
                        out=uq[:, m, :], in0=ps, scalar1=0.5,
                        scalar2=qb_sb[:, m:m + 1], op0=ALU.mult, op1=ALU.add,
                    )
            # tau: th = tanh((hid @ w2 + b2)/2), pre-broadcast via w2rep
            tauq = mpool.tile([128, W, QT], BF, tag="tauq")
            for p in range(W // 2):
                pl = ps_log.tile([128, 2 * QT], FP32, tag="logit")
                for k in range(MH):
                    nc.tensor.matmul(
                        pl,
                        w2rep_sb[:, k, :],
                        hsss[p][:, k, :, :],
                        start=(k == 0),
                        stop=(k == MH - 1),
                    )
                nc.scalar.activation(
                    out=tauq[:, 2 * p:2 * p + 2, :],
                    in_=pl.rearrange("p (a b) -> p a b", a=2),
                    func=AF.Tanh,
                    bias=b2h_sb[:, 0:1], scale=0.5,
                )
            # gate-sum gs = sum_w (1+th_w) * u'_w on DVE
            nc.vector.tensor_scalar_add(tauq, tauq, 1.0)
            gsq = gspool.tile([128, MD, QT], BF, tag="gsq")
            gsqs.append(gsq)
            pt = []
            for w in range(W):
                o = HALO - 1 - w
                t = mpool.tile([128, MD, QT], BF, tag="pt", bufs=4)
                nc.vector.tensor_mul(t, bcast(tauq, w), uq[:, :, o:o + QT])
                pt.append(t)
                if w == 1:
                    m01 = mpool.tile([128, MD, QT], BF, tag="pt", bufs=4)
                    nc.vector.tensor_add(m01, pt[0], pt[1])
            nc.vector.tensor_add(pt[3], pt[2], pt[3])
            nc.vector.tensor_add(gsq, m01, pt[3])
            if q >= 1:
                emit_D(q - 1)
        emit_D(NQ - 1)
    nc.compile()
    return nc


_CACHE: dict = {}


def _get_nc(use_gamma_beta: bool, use_merge_b: bool):
    key = (use_gamma_beta, use_merge_b)
    if key not in _CACHE:
        _CACHE[key] = build_nc(use_gamma_beta, use_merge_b)
    return _CACHE[key]


def kernel(x, w1, b1, w2, b2, wv_w, wv_b, merge_w, merge_b, gamma, beta):
    x = np.asarray(x, dtype=np.float32)
    w1 = np.asarray(w1, dtype=np.float32)
    b1 = np.asarray(b1, dtype=np.float32)
    w2 = np.asarray(w2, dtype=np.float32)
    b2 = np.asarray(b2, dtype=np.float32)
    wv_w = np.asarray(wv_w, dtype=np.float32)
    wv_b = np.asarray(wv_b, dtype=np.float32)
    merge_w = np.asarray(merge_w, dtype=np.float32)
    merge_b = np.asarray(merge_b, dtype=np.float32)
    gamma = np.asarray(gamma, dtype=np.float32)
    beta = np.asarray(beta, dtype=np.float32)

    use_gamma_beta = not (np.all(gamma == 1.0) and np.all(beta == 0.0))
    use_merge_b = bool(np.any(merge_b != 0.0))
    nc = _get_nc(use_gamma_beta, use_merge_b)

    m1f = merge_w[:D]
    m2f = merge_w[D:]
    P = wv_w @ m2f                          # fold wv and merge projections
    qb = wv_b @ m2f

    wac_h = np.ascontiguousarray(
        (np.concatenate([w1[:D], w1[D:]], axis=1) * W1SCALE)
        .reshape(KD, 128, MD, 128).transpose(1, 2, 0, 3)
    ).astype(FP8E3)
    pw_h = np.ascontiguousarray(
        P.reshape(KD, 128, MD, 128).transpose(1, 2, 0, 3)
    ).astype(BF16)
    m1_h = np.ascontiguousarray(
        m1f.reshape(KD, 128, 2, 512).transpose(1, 2, 0, 3)
    ).astype(BF16)
    w2_h = np.ascontiguousarray(
        np.broadcast_to(w2.reshape(MH, 128, 1), (MH, 128, 128)).transpose(1, 0, 2)
    ).astype(BF16)
    b1zv = np.concatenate([b1, np.zeros(D - H, np.float32)])

    shared = {
        "wac": wac_h,
        "pw": pw_h,
        "m1": m1_h,
        "w2rep": w2_h,
        "iden": np.eye(128, dtype=BF16),
        "bias": np.ascontiguousarray(np.concatenate([
            b1zv.reshape(MD, 128).T.astype(np.float32),
            (0.5 * qb).astype(np.float32).reshape(MD, 128).T,
            np.full((128, 1), 0.5 * float(b2[0]), np.float32),
        ], axis=1)),
    }
    if use_gamma_beta:
        shared["gam"] = gamma.reshape(1, D)
        shared["bet"] = beta.reshape(1, D)
    if use_merge_b:
        shared["mbt"] = merge_b.reshape(1, D)

    x2T = np.ascontiguousarray(x.reshape(B * T, D).astype(BF16).T)  # [D, B*T]
    in_maps = []
    for c in range(NCORES):
        t0 = c * NTOK
        xsT = np.zeros((D, GRID), BF16)
        xsT[:, HALO:] = x2T[:, t0:t0 + NTOK]
        if t0 % T != 0:  # halo stays inside the same batch element
            xsT[:, :HALO] = x2T[:, t0 - HALO:t0]
        xk = xsT.reshape(KD, 128, GRID)
        xq_h = np.empty((128, NQ, KD, QG), BF16)
        for q in range(NQ):
            xq_h[:, q] = xk[:, :, q * QT:q * QT + QG].transpose(1, 0, 2)
        m = dict(shared)
        m["xq"] = xq_h
        in_maps.append(m)

    res = run_bass_kernel_spmd(nc, in_maps, core_ids=list(range(NCORES)))
    out = np.concatenate([r["y"] for r in res.results], axis=0)
    return out.reshape(B, T, D).astype(np.float32)


# revision 25
# speedup vs baseline: 1.4438x; 1.0270x over previous
"""Trainium2 Bass kernel for CausalTensionGraphLayer.

Math (reference factorization, with the wv/m2 merge folded on host):
  ac  = x @ [w1a | w1c] + [b1 | 0]      [grid, D]   (chunks 0-3 = a, 4-7 = c)
  u'  = 0.5 * (x @ P + qb)              [grid, D]   P = wv_w @ merge_w[D:],
                                                    qb = wv_b @ merge_w[D:]
  hid_w  = silu(a[t] + c[t-w-1])                    (c is 0 for t-w-1 < 0)
  th_w   = tanh((hid_w @ w2 + b2)/2)                (so tau = 0.5*(1+th))
  y[t]   = x[t] @ m1 + sum_w (1+th_w[t]) * u'[t-w-1] + merge_b
  out    = LayerNorm(y) * gamma + beta

Key identities: msg @ m2 = sum_w tau_w * (vzb @ m2) because tau_w[t] is a
per-token scalar (kills a full D x D matmul phase), and sigmoid(z) =
0.5*(1+tanh(z/2)) so the whole kernel fits the silu_and_others activation
table (Silu/Tanh/Copy/Square/Identity - no mid-kernel table switch).  The
0.5 folds into the u eviction, the +1 into the scalar_tensor_tensor gate
muls: zero extra instructions.

Gate weights w1 ship as fp8e3m4 scaled x64 (error feeds only the damped tau
path; validated 3.2e-3 end-to-end) which halves the startup-critical DMA.
The 1/64 descale folds into the ac eviction's activation scale.

Sharding: data-parallel over B*T = 8192 token rows, 1024 own tokens per core
plus a 4-row causal halo (zeros at batch boundaries).  No collectives.  All
inputs are pre-arranged on host into the exact SBUF tile layouts so every
DMA is a large contiguous-per-partition transfer.

Schedule: per token quarter, A (ac matmuls) -> gating front (hs adds + Silu,
overlapping U's matmuls) -> U -> tau matmuls + Tanh -> gate-sum gs (DVE
muls, GpSimd adds), with D(q-1) (merge + LayerNorm + store) interleaved one
quarter behind so the PE never idles and HAM stays at full clock.  gs
reaches the merge PSUM banks transposed via 128x128 identity matmuls that
accumulate on top of x@m1.  Eviction collects LN statistics via activation
accumulators (Copy for sum, in-place Square for sum-of-squares) over 2-bank
PSUM tiles; rstd via bit-trick + 2 Newton steps keeps everything in one
table set.  Warm-up matmuls at t=0 lift the PE clock gate during the DMA
fill.  Output is stored bf16 (within tolerance) to halve the drain.
"""

from contextlib import ExitStack

import numpy as np
import ml_dtypes

import concourse.bass as bass
import concourse.bacc as bacc
import concourse.tile as tile
from concourse import mybir
from concourse.bass_utils import run_bass_kernel_spmd

BF16 = ml_dtypes.bfloat16
FP8E4 = ml_dtypes.float8_e4m3

B, T, D = 2, 4096, 1024
H = D // 2
W = 4
EPS = 1e-5
NCORES = 8
NTOK = (B * T) // NCORES          # 1024 own tokens per core
HALO = W                          # 4
GRID = NTOK + HALO                # 1028
NQ = 4                            # token quarters per core
QT = NTOK // NQ                   # 256 own tokens per quarter
QG = QT + HALO                    # 260 grid cols per quarter
KD = D // 128                     # 8 K-chunks over D
MH = H // 128                     # 4 M-tiles over H
MD = D // 128                     # 8 M-tiles over D
NT = QT // 128                    # 2 token tiles per quarter
W1SCALE = 64.0                    # fp8e4m3 range scaling for w1
QGP = 272                         # QG padded to a 16-byte multiple for fp8 x

FP32 = mybir.dt.float32
I32 = mybir.dt.int32
BF = mybir.dt.bfloat16
F8 = mybir.dt.float8e4
AF = mybir.ActivationFunctionType
ALU = mybir.AluOpType
AX = mybir.AxisListType


def build_nc(use_gamma_beta: bool, use_merge_b: bool):
    nc = bacc.Bacc(None, target_bir_lowering=False)

    xq = nc.dram_tensor("xq", [128, NQ, KD, QG], BF, kind="ExternalInput")
    xq8 = nc.dram_tensor("xq8", [128, NQ, KD, QGP], F8, kind="ExternalInput")
    wac = nc.dram_tensor("wac", [128, MD, KD, 128], F8, kind="ExternalInput")
    pw = nc.dram_tensor("pw", [128, MD, KD, 128], BF, kind="ExternalInput")
    m1 = nc.dram_tensor("m1", [128, 2, KD, 512], BF, kind="ExternalInput")
    w2rep = nc.dram_tensor("w2rep", [128, MH, 128], BF, kind="ExternalInput")
    iden = nc.dram_tensor("iden", [128, 128], BF, kind="ExternalInput")
    # biases packed into one tensor: cols 0-7 = [b1|0], 8-15 = qb/2, 16 = b2/2
    bias = nc.dram_tensor("bias", [128, 2 * MD + 1], FP32, kind="ExternalInput")
    if use_gamma_beta:
        gam = nc.dram_tensor("gam", [1, D], FP32, kind="ExternalInput")
        bet = nc.dram_tensor("bet", [1, D], FP32, kind="ExternalInput")
    if use_merge_b:
        mbt = nc.dram_tensor("mbt", [1, D], FP32, kind="ExternalInput")
    y = nc.dram_tensor("y", [NTOK, D], BF, kind="ExternalOutput")

    with tile.TileContext(nc) as tc, ExitStack() as ctx:
        persist = ctx.enter_context(tc.tile_pool(name="persist", bufs=1))
        acpool = ctx.enter_context(tc.tile_pool(name="acpool", bufs=2))
        gspool = ctx.enter_context(tc.tile_pool(name="gspool", bufs=2))
        mpool = ctx.enter_context(tc.tile_pool(name="mpool", bufs=2))
        opool = ctx.enter_context(tc.tile_pool(name="opool", bufs=2))
        ps_acc = ctx.enter_context(tc.tile_pool(name="ps_acc", bufs=3, space="PSUM"))
        ps_log = ctx.enter_context(tc.tile_pool(name="ps_log", bufs=1, space="PSUM"))
        ps_y = ctx.enter_context(tc.tile_pool(name="ps_y", bufs=2, space="PSUM"))

        # ---- persistent tiles (SBUF layouts match DRAM exactly) ----------
        xq_sb = persist.tile([128, NQ, KD, QG], BF, tag="xq")
        xq8_sb = persist.tile([128, NQ, KD, QGP], F8, tag="xq8")
        wac_sb = persist.tile([128, MD, KD, 128], F8, tag="wac")
        pw_sb = persist.tile([128, MD, KD, 128], BF, tag="pw")
        m1_sb = persist.tile([128, 2, KD, 512], BF, tag="m1")
        w2rep_sb = persist.tile([128, MH, 128], BF, tag="w2rep")
        iden_sb = persist.tile([128, 128], BF, tag="iden")
        bias_sb = persist.tile([128, 2 * MD + 1], FP32, tag="bias")
        b1z_sb = bias_sb[:, 0:MD]
        qb_sb = bias_sb[:, MD:2 * MD]
        b2h_sb = bias_sb[:, 2 * MD:2 * MD + 1]

        # Input DMAs split across the two HWDGE rings (sync/scalar) in PE
        # consumption order; xq quarter 0 split so A(0) can start early.
        # Most issues sit on the sync engine, which has no compute to run.
        for mc in range(4):
            nc.sync.dma_start(
                out=wac_sb[:, 2 * mc:2 * mc + 2], in_=wac[:, 2 * mc:2 * mc + 2]
            )
        nc.scalar.dma_start(out=xq8_sb[:, 0], in_=xq8[:, 0])
        nc.scalar.dma_start(out=bias_sb, in_=bias[:, :])
        nc.sync.dma_start(out=xq8_sb[:, 1], in_=xq8[:, 1])
        nc.sync.dma_start(out=w2rep_sb, in_=w2rep[:, :])
        nc.sync.dma_start(out=iden_sb, in_=iden[:, :])
        nc.scalar.dma_start(out=xq_sb[:, 0, 0:4], in_=xq[:, 0, 0:4])
        nc.scalar.dma_start(out=xq_sb[:, 0, 4:KD], in_=xq[:, 0, 4:KD])
        nc.scalar.dma_start(out=pw_sb[:, 0:2], in_=pw[:, 0:2])
        nc.sync.dma_start(out=pw_sb[:, 2:4], in_=pw[:, 2:4])
        nc.sync.dma_start(out=pw_sb[:, 4:6], in_=pw[:, 4:6])
        nc.scalar.dma_start(out=pw_sb[:, 6:8], in_=pw[:, 6:8])
        nc.scalar.dma_start(out=xq8_sb[:, 2], in_=xq8[:, 2])
        nc.scalar.dma_start(out=xq8_sb[:, 3], in_=xq8[:, 3])
        nc.sync.dma_start(out=xq_sb[:, 1], in_=xq[:, 1])
        if use_gamma_beta:
            gam_sb = persist.tile([128, D], FP32, tag="gam")
            nc.sync.dma_start(out=gam_sb, in_=gam.partition_broadcast(128))
            bet_sb = persist.tile([128, D], FP32, tag="bet")
            nc.sync.dma_start(out=bet_sb, in_=bet.partition_broadcast(128))
        if use_merge_b:
            mb_sb = persist.tile([128, D], FP32, tag="mb")
            nc.sync.dma_start(out=mb_sb, in_=mbt.partition_broadcast(128))
        nc.sync.dma_start(out=xq_sb[:, 2], in_=xq[:, 2])
        nc.sync.dma_start(out=xq_sb[:, 3], in_=xq[:, 3])
        nc.sync.dma_start(out=m1_sb[:, 0], in_=m1[:, 0])
        nc.scalar.dma_start(out=m1_sb[:, 1], in_=m1[:, 1])

        magic_sb = persist.tile([128, 1], I32, tag="magic")
        nc.vector.memset(magic_sb, 0x5F3759DF)
        one_i = persist.tile([128, 1], I32, tag="onei")
        nc.vector.memset(one_i, 1)

        # ---- HAM warm-up: dummy matmuls while the first inputs stream ----
        warm_sb = persist.tile([128, 512], BF, tag="warm")
        nc.gpsimd.memset(warm_sb, 0)
        warm_ps = ps_log.tile([128, 512], FP32, tag="logit")
        NWARM = 9
        for i in range(NWARM):
            nc.tensor.matmul(
                warm_ps, warm_sb[:, 0:128], warm_sb,
                start=(i == 0), stop=(i == NWARM - 1),
            )

        # ---- main pipeline ----------------------------------------------
        def bcast(tauq, w):
            s = tauq[:, w, :]
            return bass.AP(
                tensor=s.tensor, offset=s.offset,
                ap=[s.ap[0], [0, MD], s.ap[1]],
            )

        gsqs = []

        def emit_D(q):
            g0 = q * QT
            gsq = gsqs[q]
            srow = mpool.tile([128, NT, 2], FP32, tag="srow")
            sqs = mpool.tile([128, NT, 2], FP32, tag="sqs")
            ysb = []
            yt_dt = FP32 if use_gamma_beta else BF
            for tt in range(NT):
                yps = ps_y.tile([128, 2, 512], FP32, tag="y")
                yt = opool.tile([128, 2, 512], yt_dt, tag="ysb")
                ysb.append(yt)
                for half in range(2):
                    for k in range(KD):
                        nc.tensor.matmul(
                            yps[:, half, :],
                            xq_sb[:, q, k, HALO + 128 * tt:HALO + 128 * tt + 128],
                            m1_sb[:, half, k, :],
                            start=(k == 0),
                            stop=False,
                        )
                    # gs arrives transposed via identity matmuls, accumulated
                    # into the same banks (gated message + x@m1 in one go).
                    for mm in range(4):
                        m = half * 4 + mm
                        nc.tensor.matmul(
                            yps[:, half, mm * 128:(mm + 1) * 128],
                            gsq[:, m, 128 * tt:128 * tt + 128],
                            iden_sb,
                            start=False,
                            stop=(mm == 3),
                        )
                    if use_merge_b:
                        nc.vector.tensor_add(
                            yps[:, half, :], yps[:, half, :],
                            mb_sb[:, 512 * half:512 * half + 512],
                        )
                    # per-half eviction pipelines with the PE at bank level
                    nc.scalar.activation(
                        out=yt[:, half, :], in_=yps[:, half, :], func=AF.Copy,
                        accum_out=srow[:, tt, half:half + 1],
                    )
                    nc.scalar.activation(   # in-place: this bank dies here
                        out=yps[:, half, :], in_=yps[:, half, :], func=AF.Square,
                        accum_out=sqs[:, tt, half:half + 1],
                    )
            # LayerNorm finalize; rstd via bit-trick seed + 1 Newton step.
            ssum = mpool.tile([128, NT], FP32, tag="ssum")
            nc.vector.reduce_sum(out=ssum, in_=srow, axis=AX.X)
            qsum = mpool.tile([128, NT], FP32, tag="qsum")
            nc.vector.reduce_sum(out=qsum, in_=sqs, axis=AX.X)
            mean = mpool.tile([128, NT], FP32, tag="mean")
            nc.vector.tensor_scalar_mul(mean, ssum, 1.0 / D)
            m2e = mpool.tile([128, NT], FP32, tag="m2e")
            nc.vector.scalar_tensor_tensor(   # mean^2 - eps
                out=m2e, in0=mean, scalar=1.0, in1=mean,
                op0=ALU.mult, op1=ALU.mult,
            )
            nc.vector.tensor_scalar_add(m2e, m2e, -EPS)
            veps = mpool.tile([128, NT], FP32, tag="veps")
            nc.vector.scalar_tensor_tensor(   # q/D - (mean^2 - eps)
                out=veps, in0=qsum, scalar=1.0 / D, in1=m2e,
                op0=ALU.mult, op1=ALU.subtract,
            )
            rbits = mpool.tile([128, NT], I32, tag="rbits")
            nc.vector.tensor_scalar(
                out=rbits, in0=veps.bitcast(I32), scalar1=one_i[:, 0:1],
                scalar2=None, op0=ALU.arith_shift_right,
            )
            nc.vector.tensor_tensor(
                out=rbits, in0=magic_sb.to_broadcast([128, NT]), in1=rbits,
                op=ALU.subtract,
            )
            rstd = rbits.bitcast(FP32)
            for _ in range(1):
                nt1 = mpool.tile([128, NT], FP32, tag="nt1")
                nc.vector.tensor_mul(nt1, rstd, rstd)
                nc.vector.tensor_mul(nt1, nt1, veps)
                nc.vector.tensor_scalar(
                    out=nt1, in0=nt1, scalar1=-0.5, scalar2=1.5,
                    op0=ALU.mult, op1=ALU.add,
                )
                nc.vector.tensor_mul(rstd, rstd, nt1)
            for tt in range(NT):
                tok0 = g0 + 128 * tt
                ybf = opool.tile([128, D], BF, tag="ybf")
                ytf = ysb[tt].rearrange("p a b -> p (a b)")
                if use_gamma_beta:
                    nc.vector.tensor_scalar(
                        out=ytf, in0=ytf, scalar1=mean[:, tt:tt + 1],
                        scalar2=rstd[:, tt:tt + 1],
                        op0=ALU.subtract, op1=ALU.mult,
                    )
                    nc.vector.tensor_mul(ytf, ytf, gam_sb)
                    nc.vector.tensor_add(ybf, ytf, bet_sb)
                else:
                    nc.vector.tensor_scalar(
                        out=ybf, in0=ytf, scalar1=mean[:, tt:tt + 1],
                        scalar2=rstd[:, tt:tt + 1],
                        op0=ALU.subtract, op1=ALU.mult,
                    )
                nc.sync.dma_start(out=y[tok0:tok0 + 128, :], in_=ybf)

        for q in range(NQ):
            # A(q): ac = (x8 @ [w1a|w1c]*64) / 64 + [b1|0], fp8 DoubleRow
            # matmuls contract two 128-row K-chunks per instruction.
            acq = acpool.tile([128, MD, QG], BF, tag="acq")
            for m in range(MD):
                ps = ps_acc.tile([128, QG], FP32, tag="acc")
                for k2 in range(KD // 2):
                    nc.tensor.matmul(
                        ps,
                        wac_sb[:, m, 2 * k2:2 * k2 + 2, :],
                        xq8_sb[:, q, 2 * k2:2 * k2 + 2, 0:QG],
                        start=(k2 == 0),
                        stop=(k2 == KD // 2 - 1),
                        perf_mode=mybir.MatmulPerfMode.DoubleRow,
                    )
                nc.scalar.activation(
                    out=acq[:, m, :], in_=ps, func=AF.Identity,
                    bias=b1z_sb[:, m:m + 1], scale=1.0 / W1SCALE,
                )
            # gating front: hs = a + shift(c); Silu on ScalarE overlaps U
            hsss = []
            for p in range(W // 2):
                hs = mpool.tile([128, MH, 2, QT], BF, tag="hs")
                for wi in range(2):
                    w = 2 * p + wi
                    o = HALO - 1 - w
                    nc.vector.tensor_add(
                        hs[:, :, wi, :],
                        acq[:, 0:MH, HALO:HALO + QT],
                        acq[:, MH:MD, o:o + QT],
                    )
                hss = mpool.tile([128, MH, 2, QT], BF, tag="hss")
                nc.scalar.activation(out=hss, in_=hs, func=AF.Silu)
                hsss.append(hss)
            # U(q): u' = 0.5*(x @ P + qb) on the quarter grid
            uq = acpool.tile([128, MD, QG], BF, tag="uq")
            for m in range(MD):
                ps = ps_acc.tile([128, QG], FP32, tag="acc")
                for k in range(KD):
                    nc.tensor.matmul(
                        ps,
                        pw_sb[:, m, k, :],
                        xq_sb[:, q, k, :],
                        start=(k == 0),
                        stop=(k == KD - 1),
                    )
                if m < 4:   # split evictions across ScalarE and DVE
                    nc.scalar.activation(
                        out=uq[:, m, :], in_=ps, func=AF.Identity,
                        bias=qb_sb[:, m:m + 1], scale=0.5,
                    )
                else:
                    nc.vector.tensor_scalar(
                        out=uq[:, m, :], in0=ps, scalar1=0.5,
                        scalar2=qb_sb[:, m:m + 1], op0=ALU.mult, op1=ALU.add,
                    )
            # tau: th = tanh((hid @ w2 + b2)/2), pre-broadcast via w2rep
            tauq = mpool.tile([128, W, QT], BF, tag="tauq")
            for p in range(W // 2):
                pl = ps_log.tile([128, 2 * QT], FP32, tag="logit")
                for k in range(MH):
                    nc.tensor.matmul(
                        pl,
                        w2rep_sb[:, k, :],
                        hsss[p][:, k, :, :],
                        start=(k == 0),
                        stop=(k == MH - 1),
                    )
                nc.scalar.activation(
                    out=tauq[:, 2 * p:2 * p + 2, :],
                    in_=pl.rearrange("p (a b) -> p a b", a=2),
                    func=AF.Tanh,
                    bias=b2h_sb[:, 0:1], scale=0.5,
                )
            # gate-sum gs = sum_w (1+th_w) * u'_w on DVE
            nc.vector.tensor_scalar_add(tauq, tauq, 1.0)
            gsq = gspool.tile([128, MD, QT], BF, tag="gsq")
            gsqs.append(gsq)
            pt = []
            for w in range(W):
                o = HALO - 1 - w
                t = mpool.tile([128, MD, QT], BF, tag="pt", bufs=4)
                nc.vector.tensor_mul(t, bcast(tauq, w), uq[:, :, o:o + QT])
                pt.append(t)
                if w == 1:
                    m01 = mpool.tile([128, MD, QT], BF, tag="pt", bufs=4)
                    nc.vector.tensor_add(m01, pt[0], pt[1])
            nc.vector.tensor_add(pt[3], pt[2], pt[3])
            nc.vector.tensor_add(gsq, m01, pt[3])
            if q >= 1:
                emit_D(q - 1)
        emit_D(NQ - 1)
    nc.compile()
    return nc


_CACHE: dict = {}


def _get_nc(use_gamma_beta: bool, use_merge_b: bool):
    key = (use_gamma_beta, use_merge_b)
    if key not in _CACHE:
        _CACHE[key] = build_nc(use_gamma_beta, use_merge_b)
    return _CACHE[key]


def kernel(x, w1, b1, w2, b2, wv_w, wv_b, merge_w, merge_b, gamma, beta):
    x = np.asarray(x, dtype=np.float32)
    w1 = np.asarray(w1, dtype=np.float32)
    b1 = np.asarray(b1, dtype=np.float32)
    w2 = np.asarray(w2, dtype=np.float32)
    b2 = np.asarray(b2, dtype=np.float32)
    wv_w = np.asarray(wv_w, dtype=np.float32)
    wv_b = np.asarray(wv_b, dtype=np.float32)
    merge_w = np.asarray(merge_w, dtype=np.float32)
    merge_b = np.asarray(merge_b, dtype=np.float32)
    gamma = np.asarray(gamma, dtype=np.float32)
    beta = np.asarray(beta, dtype=np.float32)

    use_gamma_beta = not (np.all(gamma == 1.0) and np.all(beta == 0.0))
    use_merge_b = bool(np.any(merge_b != 0.0))
    nc = _get_nc(use_gamma_beta, use_merge_b)

    m1f = merge_w[:D]
    m2f = merge_w[D:]
    P = wv_w @ m2f                          # fold wv and merge projections
    qb = wv_b @ m2f

    wac_h = np.ascontiguousarray(
        (np.concatenate([w1[:D], w1[D:]], axis=1) * W1SCALE)
        .reshape(KD, 128, MD, 128).transpose(1, 2, 0, 3)
    ).astype(FP8E4)
    pw_h = np.ascontiguousarray(
        P.reshape(KD, 128, MD, 128).transpose(1, 2, 0, 3)
    ).astype(BF16)
    m1_h = np.ascontiguousarray(
        m1f.reshape(KD, 128, 2, 512).transpose(1, 2, 0, 3)
    ).astype(BF16)
    w2_h = np.ascontiguousarray(
        np.broadcast_to(w2.reshape(MH, 128, 1), (MH, 128, 128)).transpose(1, 0, 2)
    ).astype(BF16)
    b1zv = np.concatenate([b1, np.zeros(D - H, np.float32)])

    shared = {
        "wac": wac_h,
        "pw": pw_h,
        "m1": m1_h,
        "w2rep": w2_h,
        "iden": np.eye(128, dtype=BF16),
        "bias": np.ascontiguousarray(np.concatenate([
            b1zv.reshape(MD, 128).T.astype(np.float32),
            (0.5 * qb).astype(np.float32).reshape(MD, 128).T,
            np.full((128, 1), 0.5 * float(b2[0]), np.float32),
        ], axis=1)),
    }
    if use_gamma_beta:
        shared["gam"] = gamma.reshape(1, D)
        shared["bet"] = beta.reshape(1, D)
    if use_merge_b:
        shared["mbt"] = merge_b.reshape(1, D)

    x2 = x.reshape(B * T, D)
    x2T = np.ascontiguousarray(x2.astype(BF16).T)      # [D, B*T] bf16
    x2T8 = np.ascontiguousarray(x2.astype(FP8E4).T)    # [D, B*T] fp8 (gates)
    in_maps = []
    for c in range(NCORES):
        t0 = c * NTOK
        xsT = np.zeros((D, GRID), BF16)
        xsT[:, HALO:] = x2T[:, t0:t0 + NTOK]
        xsT8 = np.zeros((D, GRID), FP8E4)
        xsT8[:, HALO:] = x2T8[:, t0:t0 + NTOK]
        if t0 % T != 0:  # halo stays inside the same batch element
            xsT[:, :HALO] = x2T[:, t0 - HALO:t0]
            xsT8[:, :HALO] = x2T8[:, t0 - HALO:t0]
        xk = xsT.reshape(KD, 128, GRID)
        xk8 = xsT8.reshape(KD, 128, GRID)
        xq_h = np.empty((128, NQ, KD, QG), BF16)
        xq8_h = np.zeros((128, NQ, KD, QGP), FP8E4)
        for q in range(NQ):
            xq_h[:, q] = xk[:, :, q * QT:q * QT + QG].transpose(1, 0, 2)
            xq8_h[:, q, :, 0:QG] = xk8[:, :, q * QT:q * QT + QG].transpose(1, 0, 2)
        m = dict(shared)
        m["xq"] = xq_h
        m["xq8"] = xq8_h
        in_maps.append(m)

    res = run_bass_kernel_spmd(nc, in_maps, core_ids=list(range(NCORES)))
    out = np.concatenate([r["y"] for r in res.results], axis=0)
    return out.reshape(B, T, D).astype(np.float32)
